# revision 32
# baseline (speedup 1.0000x reference)
"""Bidirectional tanh-RNN (B=32, S=512, I=H=1024) on 8 Trainium2 NeuronCores.

Sharding: 2 direction groups x 4 cores. Cores 0-3 run the forward RNN,
cores 4-7 the backward RNN (host reverses time for them, so the device
kernel is direction-agnostic). Within a group the batch (32) is split 4
ways -> 8 sequences per core. Weights are replicated per direction.

Per-core kernel (all fp32 storage, float32r matmuls):
  - Input projection xp = x @ W_ih.T + bias is computed in chunks of 16
    steps as [128 (16 steps x 8 batch), 1024] PSUM tiles, interleaved with
    the recurrence so it hides in recurrence dependency-chain bubbles.
  - A DMA "reshuffle" moves xp from [16s*8b, H] PSUM layout into an
    [8b, steps*H] SBUF layout consumable by the recurrence.
  - Recurrence step (h-stationary): PSUM[8, 512]x2 accumulates
    I8.T @ xp_t  (xp add via identity matmul)  +
    sum_k hT_k.T @ W_hhT[k-tile]   (state stationary, W streamed).
    Then tanh (ACT) -> h halves, PE-transpose h -> hT tiles (PSUM), one
    DVE copy -> SBUF for the next step's stationary operands.
  - h is DMA'd to DRAM as out[8t+b, :].
"""

import numpy as np
from contextlib import ExitStack

import concourse.bass as bass
import concourse.mybir as mybir
import concourse.tile as tile
from concourse import bacc
from concourse.masks import make_identity

F32 = mybir.dt.float32
F32R = mybir.dt.float32r

B, S, I, H = 32, 512, 1024, 1024
NCORES = 8
BL = 8          # local batch per core
KT = 8          # 128-row contraction tiles over I/H
CH = 16         # steps per projection chunk (M-tile of 128 = 16*8 rows)
HF = 8          # steps per xp half-chunk buffer


def _emit_body(ctx: ExitStack, tc: tile.TileContext, xT, w, u, bias, out, steps):
    nc = tc.nc
    n_chunks = steps // CH
    n_halves = steps // HF
    assert steps % CH == 0

    const = ctx.enter_context(tc.tile_pool(name="const", bufs=1))
    xpool = ctx.enter_context(tc.tile_pool(name="xc", bufs=2))
    ppool = ctx.enter_context(tc.tile_pool(name="proj", bufs=1, space="PSUM"))
    rpool = ctx.enter_context(tc.tile_pool(name="rec", bufs=2, space="PSUM"))
    xbpool = ctx.enter_context(tc.tile_pool(name="xb", bufs=8))
    pspool = ctx.enter_context(tc.tile_pool(name="pjs", bufs=2))
    htpool2 = ctx.enter_context(tc.tile_pool(name="ht2", bufs=2))
    dpool = ctx.enter_context(tc.tile_pool(name="dram", bufs=1, space="DRAM"))
    hpool = ctx.enter_context(tc.tile_pool(name="h", bufs=2))
    htpool = ctx.enter_context(tc.tile_pool(name="ht", bufs=2))
    tfpool = ctx.enter_context(tc.tile_pool(name="tf", bufs=2))
    tfpool2 = ctx.enter_context(tc.tile_pool(name="tf2", bufs=2))

    # --- constants / resident weights ---
    w_sb = const.tile([128, KT, H], F32R)
    nc.gpsimd.dma_start(w_sb[:], w.rearrange("(k p) n -> p k n", p=128))
    u_sb = const.tile([128, KT, H], F32R)
    nc.gpsimd.dma_start(u_sb[:], u.rearrange("(k p) n -> p k n", p=128))
    bias_sb = const.tile([1, H], F32R)
    nc.gpsimd.dma_start(bias_sb[:], bias[:])
    ones1_f = const.tile([1, 128], F32)
    nc.gpsimd.memset(ones1_f[:], 1.0)
    ones1 = const.tile([1, 128], F32R)
    nc.vector.tensor_copy(ones1[:], ones1_f[:])
    # [8, 32] identity-padded-with-zeros: the xp-add matmul broadcasts xp
    # into rows 0-7 AND zero-fills rows 8-31 of the PSUM tile so every row
    # the stream-transpose later reads is initialized.
    ident8_f = const.tile([8, 32], F32)
    nc.gpsimd.memset(ident8_f[:], 0.0)
    nc.gpsimd.affine_select(
        out=ident8_f[:], in_=ident8_f[:],
        compare_op=mybir.AluOpType.not_equal, fill=1.0, base=0,
        pattern=[[-1, 32]], channel_multiplier=1)
    ident8 = const.tile([8, 32], F32R)
    nc.vector.tensor_copy(ident8[:], ident8_f[:])

    xT_r = xT.rearrange("(k p) m -> p k m", p=128)
    xp_dram = dpool.tile([steps * BL, H], F32R)

    from collections import deque
    proj_pending = deque()

    def queue_proj(c):
        """Queue chunk c's projection matmuls as bubble-filler thunks."""
        xc = xpool.tile([128, KT, CH * BL], F32R, tag="xc")
        nc.gpsimd.dma_start(xc[:], xT_r[:, :, c * CH * BL:(c + 1) * CH * BL])
        xs_sb = pspool.tile([128, H], F32R, tag="pjs")
        state = {}

        def mk(b, k):
            nbs = slice(512 * b, 512 * (b + 1))

            def run():
                if k == -1:
                    p = ppool.tile([128, 512], F32, tag=f"pj{b}")
                    state[b] = p
                    nc.tensor.matmul(p[:], lhsT=ones1[:],
                                     rhs=bias_sb[:, nbs],
                                     start=True, stop=False)
                    return
                p = state[b]
                nc.tensor.matmul(p[:], lhsT=xc[:, k, :],
                                 rhs=w_sb[:, k, nbs],
                                 start=False, stop=(k == KT - 1))
                if k == KT - 1:
                    nc.scalar.activation(xs_sb[:, nbs], p[:],
                                         mybir.ActivationFunctionType.Copy)
                    if b == 1:
                        nc.gpsimd.dma_start(
                            xp_dram[128 * c:128 * (c + 1), :], xs_sb[:])
            return run

        for b in range(2):
            for k in range(-1, KT):
                proj_pending.append(mk(b, k))

    def drain_proj(n):
        for _ in range(min(n, len(proj_pending))):
            proj_pending.popleft()()

    LEAD = 6

    def emit_xb_fetch(t):
        xb = xbpool.tile([BL, H], F32R, tag="xb")
        nc.gpsimd.dma_start(xb[:], xp_dram[BL * t:BL * (t + 1), :])
        return xb

    # --- prologue: chunk 0 projection fully, then first fetches ---
    queue_proj(0)
    drain_proj(99)
    xb_tiles = {t: emit_xb_fetch(t) for t in range(LEAD)}

    prev_halves = None
    ht_lo = ht_hi = None
    for t in range(steps):
        c, j = divmod(t, CH)
        if j == 0 and c + 1 < n_chunks:
            queue_proj(c + 1)
        if t + LEAD < steps:
            xb_tiles[t + LEAD] = emit_xb_fetch(t + LEAD)
        xb = xb_tiles.pop(t)

        # xp-add matmuls open this step's PSUM accumulation groups; they only
        # need xb, so they fill the PE while the previous tanh/transpose runs.
        # The padded identity also zero-fills rows 8-31.
        rec = []
        for b in range(2):
            nbs = slice(512 * b, 512 * (b + 1))
            r = rpool.tile([32, 512], F32, tag=f"rec{b}")
            nc.tensor.matmul(r[:], lhsT=ident8[:],
                             rhs=xb[:, nbs],
                             start=True, stop=(t == 0))
            rec.append(r)

        drain_proj(2)

        if t > 0:
            # DVE 32x32 stream-transposes: h halves (j-major permuted columns,
            # see tanh below) -> hT staging, then one rounding copy
            # (f32 -> f32r) per half for the matmul stationaries.
            ht_rs = []
            for half in range(2):
                tf = (tfpool if half == 0 else tfpool2).tile(
                    [128, 128], F32, tag=f"tf{half}")
                for j in range(4):
                    src = prev_halves[half][j // 2]
                    nc.vector.transpose(
                        tf[32 * j:32 * (j + 1), :],
                        src[0:32, 128 * (j % 2):128 * (j % 2 + 1)])
                hpool_r = htpool if half == 0 else htpool2
                ht_r = hpool_r.tile([128, 128], F32R, tag=f"ht{half}")
                nc.vector.tensor_copy(ht_r[:], tf[:])
                ht_rs.append(ht_r)
            ht_lo, ht_hi = ht_rs

            # bank 0 fully first so its tanh (which gates the next step's
            # lo-half transposes) fires ~2 us before the period ends.
            for b in range(2):
                nbs = slice(512 * b, 512 * (b + 1))
                for k in range(KT):
                    src = ht_lo if k < 4 else ht_hi
                    if k == KT - 1:
                        # 32-col stationary: cols 8-31 hold transposed zero
                        # rows, so rows 8-31 get += 0 and stop=True closes
                        # the whole [32, 512] accumulation group.
                        nc.tensor.matmul(
                            rec[b][:],
                            lhsT=src[:, 96:128],
                            rhs=u_sb[:, k, nbs],
                            start=False, stop=True)
                    else:
                        nc.tensor.matmul(
                            rec[b][0:BL, :],
                            lhsT=src[:, 32 * (k % 4):32 * (k % 4) + 8],
                            rhs=u_sb[:, k, nbs],
                            start=False, stop=False)

        # --- tanh ---
        # Output columns are written j-major permuted (logical col 128i+32j+b
        # lands at 128j+32i+b) so each stream-transpose above reads one
        # contiguous [32, 128] range. The host un-permutes (see assemble()).
        # Two separate tiles per bank (j-pairs) so the first transposes are
        # gated only by the first tanh instruction.
        halves = []
        for b in range(2):
            src = rec[b][:].rearrange("p (i j b) -> p i j b", i=4, j=4)
            sub = []
            for jj in range(2):
                hh_t = hpool.tile([32, 256], F32, tag=f"h{b}{jj}")
                nc.scalar.activation(
                    hh_t[:].rearrange("p (j i b) -> p i j b", j=2, i=4),
                    src[:, :, 2 * jj:2 * (jj + 1), :],
                    mybir.ActivationFunctionType.Tanh)
                sub.append(hh_t)
            halves.append(sub)

        # --- output DMA ---
        for b in range(2):
            for jj in range(2):
                nc.sync.dma_start(
                    out[BL * t:BL * (t + 1),
                        512 * b + 256 * jj:512 * b + 256 * (jj + 1)],
                    halves[b][jj][0:BL, :])

        prev_halves = halves


def build_nc(steps=S, enable_asserts=False):
    nc = bacc.Bacc("TRN2", target_bir_lowering=False, debug=False,
                   enable_asserts=enable_asserts)
    xT = nc.dram_tensor("xT", [I, steps * BL], F32R, kind="ExternalInput").ap()
    w = nc.dram_tensor("w", [I, H], F32R, kind="ExternalInput").ap()
    u = nc.dram_tensor("u", [H, H], F32R, kind="ExternalInput").ap()
    bias = nc.dram_tensor("bias", [1, H], F32R, kind="ExternalInput").ap()
    out = nc.dram_tensor("out", [steps * BL, H], F32, kind="ExternalOutput").ap()
    with tile.TileContext(nc) as tc:
        with ExitStack() as ctx:
            _emit_body(ctx, tc, xT, w, u, bias, out, steps)
    nc.compile()
    return nc


def round_f32r(a):
    """Round fp32 to the FP32R format (11 mantissa bits, RNE, low 12 bits 0)."""
    u = np.ascontiguousarray(a, dtype=np.float32).view(np.uint32)
    u = u + np.uint32(0x7FF) + ((u >> np.uint32(12)) & np.uint32(1))
    u &= np.uint32(0xFFFFF000)
    return u.view(np.float32)


def make_in_maps(x, W_ih_f, W_hh_f, b_ih_f, b_hh_f, W_ih_b, W_hh_b, b_ih_b, b_hh_b,
                 steps=S):
    """Build the 8 per-core input dicts. Cores 0-3 fwd, 4-7 bwd."""
    x = np.ascontiguousarray(np.asarray(x, dtype=np.float32)[:, :steps])
    sets = {
        "f": (np.asarray(W_ih_f), np.asarray(W_hh_f),
              np.asarray(b_ih_f) + np.asarray(b_hh_f)),
        "b": (np.asarray(W_ih_b), np.asarray(W_hh_b),
              np.asarray(b_ih_b) + np.asarray(b_hh_b)),
    }
    in_maps = []
    for core in range(NCORES):
        d = "f" if core < 4 else "b"
        g = core % 4
        Wih, Whh, bsum = sets[d]
        xs = x[BL * g:BL * (g + 1)]
        if d == "b":
            xs = xs[:, ::-1]
        # xT[i, s*BL + b] = xs[b, s, i]
        xT = np.ascontiguousarray(xs.transpose(2, 1, 0).reshape(I, steps * BL))
        in_maps.append({
            "xT": round_f32r(xT),
            "w": round_f32r(np.ascontiguousarray(Wih.T.astype(np.float32))),
            "u": round_f32r(np.ascontiguousarray(Whh.T.astype(np.float32))),
            "bias": round_f32r(np.ascontiguousarray(bsum.astype(np.float32)[None, :])),
        })
    return in_maps


def unpermute_out(o):
    """Undo the device-side j-major column permutation (per 512-col half)."""
    s = o.shape[:-1]
    return np.ascontiguousarray(
        o.reshape(*s, 2, 4, 4, 32).swapaxes(-3, -2).reshape(*s, H))


def assemble(results, steps=S):
    """results: list of 8 dicts with 'out' [steps*BL, H]. Returns [B, steps, 2H]."""
    full = np.empty((B, steps, 2 * H), dtype=np.float32)
    for core in range(NCORES):
        o = unpermute_out(np.asarray(results[core]["out"])).reshape(steps, BL, H)
        g = core % 4
        if core < 4:
            full[BL * g:BL * (g + 1), :, :H] = o.transpose(1, 0, 2)
        else:
            full[BL * g:BL * (g + 1), :, H:] = o[::-1].transpose(1, 0, 2)
    return full


def kernel(x, W_ih_f, W_hh_f, b_ih_f, b_hh_f, W_ih_b, W_hh_b, b_ih_b, b_hh_b):
    from concourse.bass_utils import run_bass_kernel_spmd
    nc = build_nc(S)
    in_maps = make_in_maps(x, W_ih_f, W_hh_f, b_ih_f, b_hh_f,
                           W_ih_b, W_hh_b, b_ih_b, b_hh_b)
    res = run_bass_kernel_spmd(nc, in_maps, list(range(NCORES))).results
    return assemble(res)


# revision 34
# speedup vs baseline: 1.2779x; 1.2779x over previous
"""Bidirectional tanh-RNN (B=32, S=512, I=H=1024) on 8 Trainium2 NeuronCores.

Sharding: 2 direction groups x 4 cores. Cores 0-3 run the forward RNN,
cores 4-7 the backward RNN (host reverses time for them, so the device
kernel is direction-agnostic). Within a group the batch (32) is split 4
ways -> 8 sequences per core. Weights are replicated per direction.

Per-core kernel (fp32 storage, float32r matmuls, h-stationary recurrence):
  - Input projection xp = x @ W_ih.T + bias runs in 16-step chunks as
    [128 (16s x 8b), 1024] PSUM tiles, its matmuls spread one-or-two per
    step to fill recurrence dependency bubbles; results stage through
    DRAM and are prefetched per-step as [8, 1024] xb tiles.
  - Per step, xp is ACT-copied into rows 0-7 of one of 4 persistent
    [32, 512] recurrence PSUM tiles (2 banks x 2 step parities) whose
    has_written bits were set once by prologue dummy matmul groups; the
    16 start=False u-matmuls (lhsT = hT k-tiles, rhs = streamed W_hhT)
    then accumulate onto the copied xp.
  - tanh (ACT, split in 4 so chains start early) writes h with j-major
    permuted columns; DVE 32x32 stream-transposes (cross-partition-base)
    rebuild hT, and one DVE copy per half rounds f32 -> f32r for the next
    step's stationary operands. The host un-permutes the output columns.
  - h rows are DMA'd to DRAM as out[8t+b, :].
"""

import numpy as np
from contextlib import ExitStack

import concourse.bass as bass
import concourse.mybir as mybir
import concourse.tile as tile
from concourse import bacc
from concourse.masks import make_identity

F32 = mybir.dt.float32
F32R = mybir.dt.float32r

B, S, I, H = 32, 512, 1024, 1024
NCORES = 8
BL = 8          # local batch per core
KT = 8          # 128-row contraction tiles over I/H
CH = 16         # steps per projection chunk (M-tile of 128 = 16*8 rows)
HF = 8          # steps per xp half-chunk buffer


def _emit_body(ctx: ExitStack, tc: tile.TileContext, xT, w, u, bias, out, steps):
    nc = tc.nc
    n_chunks = steps // CH
    n_halves = steps // HF
    assert steps % CH == 0

    const = ctx.enter_context(tc.tile_pool(name="const", bufs=1))
    xpool = ctx.enter_context(tc.tile_pool(name="xc", bufs=2))
    ppool = ctx.enter_context(tc.tile_pool(name="proj", bufs=1, space="PSUM"))
    rpool = ctx.enter_context(tc.tile_pool(name="rec", bufs=1, space="PSUM"))
    xbpool = ctx.enter_context(tc.tile_pool(name="xb", bufs=8))
    pspool = ctx.enter_context(tc.tile_pool(name="pjs", bufs=2))
    htpool2 = ctx.enter_context(tc.tile_pool(name="ht2", bufs=2))
    dpool = ctx.enter_context(tc.tile_pool(name="dram", bufs=1, space="DRAM"))
    hpool = ctx.enter_context(tc.tile_pool(name="h", bufs=2))
    htpool = ctx.enter_context(tc.tile_pool(name="ht", bufs=2))
    tfpool = ctx.enter_context(tc.tile_pool(name="tf", bufs=2))
    tfpool2 = ctx.enter_context(tc.tile_pool(name="tf2", bufs=2))

    # --- constants / resident weights ---
    w_sb = const.tile([128, KT, H], F32R)
    nc.gpsimd.dma_start(w_sb[:], w.rearrange("(k p) n -> p k n", p=128))
    u_sb = const.tile([128, KT, H], F32R)
    nc.gpsimd.dma_start(u_sb[:], u.rearrange("(k p) n -> p k n", p=128))
    bias_sb = const.tile([1, H], F32R)
    nc.gpsimd.dma_start(bias_sb[:], bias[:])
    ones1_f = const.tile([1, 128], F32)
    nc.gpsimd.memset(ones1_f[:], 1.0)
    ones1 = const.tile([1, 128], F32R)
    nc.vector.tensor_copy(ones1[:], ones1_f[:])
    # [1, 32] mask (0 in cols 0-7, 1 in cols 8-31): gives the t=0 tile an
    # owned bounded write on rows 8-31 without touching the xp rows.
    zmask_f = const.tile([1, 32], F32)
    nc.gpsimd.memset(zmask_f[:], 1.0)
    nc.gpsimd.memset(zmask_f[:, 0:8], 0.0)
    zmask = const.tile([1, 32], F32R)
    nc.vector.tensor_copy(zmask[:], zmask_f[:])

    xT_r = xT.rearrange("(k p) m -> p k m", p=128)
    xp_dram = dpool.tile([steps * BL, H], F32)

    from collections import deque
    proj_pending = deque()

    def queue_proj(c):
        """Queue chunk c's projection matmuls as bubble-filler thunks."""
        xc = xpool.tile([128, KT, CH * BL], F32R, tag="xc")
        nc.gpsimd.dma_start(xc[:], xT_r[:, :, c * CH * BL:(c + 1) * CH * BL])
        xs_sb = pspool.tile([128, H], F32, tag="pjs")
        state = {}

        def mk(b, k):
            nbs = slice(512 * b, 512 * (b + 1))

            def run():
                if k == -1:
                    p = ppool.tile([128, 512], F32, tag=f"pj{b}")
                    state[b] = p
                    nc.tensor.matmul(p[:], lhsT=ones1[:],
                                     rhs=bias_sb[:, nbs],
                                     start=True, stop=False)
                    return
                p = state[b]
                nc.tensor.matmul(p[:], lhsT=xc[:, k, :],
                                 rhs=w_sb[:, k, nbs],
                                 start=False, stop=(k == KT - 1))
                if k == KT - 1:
                    nc.scalar.activation(xs_sb[:, nbs], p[:],
                                         mybir.ActivationFunctionType.Copy)
                    if b == 1:
                        nc.gpsimd.dma_start(
                            xp_dram[128 * c:128 * (c + 1), :], xs_sb[:])
            return run

        for b in range(2):
            for k in range(-1, KT):
                proj_pending.append(mk(b, k))

    def drain_proj(n):
        for _ in range(min(n, len(proj_pending))):
            proj_pending.popleft()()

    LEAD = 6

    def emit_xb_fetch(t):
        xb = xbpool.tile([BL, H], F32, tag="xb")
        nc.gpsimd.dma_start(xb[:], xp_dram[BL * t:BL * (t + 1), :])
        return xb

    # --- prologue: chunk 0 projection fully, then first fetches ---
    queue_proj(0)
    drain_proj(99)
    xb_tiles = {t: emit_xb_fetch(t) for t in range(LEAD)}

    # 4 persistent recurrence PSUM tiles (2 banks x 2 step-parities).
    # One closed dummy matmul group each sets the has_written bits (and
    # initializes rows 8-31 with bounded constants); the bits persist, so
    # the per-step start=False matmuls accumulate onto ACT-copied xp.
    rec_t = {}
    for par in range(2):
        for b in range(2):
            r = rpool.tile([32, 512], F32, tag=f"rec{b}{par}")
            nc.tensor.matmul(r[:], lhsT=ones1[:, 0:32],
                             rhs=bias_sb[0:1, 0:512], start=True, stop=True)
            rec_t[(b, par)] = r

    def emit_xp_copy(t):
        """ACT-copy step t's xp into rows 0-7 of its parity tiles."""
        xbt = xb_tiles.pop(t)
        tiles = [rec_t[(b, t % 2)] for b in range(2)]
        for b in range(2):
            nbs = slice(512 * b, 512 * (b + 1))
            nc.scalar.activation(tiles[b][0:BL, :], xbt[:, nbs],
                                 mybir.ActivationFunctionType.Copy)
        return tiles

    rec_cur = emit_xp_copy(0)

    prev_halves = None
    ht_lo = ht_hi = None
    for t in range(steps):
        c, j = divmod(t, CH)
        if j == 0 and c + 1 < n_chunks:
            queue_proj(c + 1)
        if t + LEAD < steps:
            xb_tiles[t + LEAD] = emit_xb_fetch(t + LEAD)
        rec = rec_cur

        drain_proj(2)

        if t > 0:
            # DVE 32x32 stream-transposes: h halves (j-major permuted columns,
            # see tanh below) -> hT staging, then one rounding copy
            # (f32 -> f32r) per half for the matmul stationaries.
            ht_rs = []
            for half in range(2):
                tf = (tfpool if half == 0 else tfpool2).tile(
                    [128, 128], F32, tag=f"tf{half}")
                for j in range(4):
                    src = prev_halves[half][j // 2]
                    nc.vector.transpose(
                        tf[32 * j:32 * (j + 1), :],
                        src[0:32, 128 * (j % 2):128 * (j % 2 + 1)])
                hpool_r = htpool if half == 0 else htpool2
                ht_r = hpool_r.tile([128, 128], F32R, tag=f"ht{half}")
                nc.vector.tensor_copy(ht_r[:], tf[:])
                ht_rs.append(ht_r)
            ht_lo, ht_hi = ht_rs

            # bank 0 fully first so its tanh (which gates the next step's
            # lo-half transposes) fires ~2 us before the period ends.
            for b in range(2):
                nbs = slice(512 * b, 512 * (b + 1))
                for k in range(KT):
                    src = ht_lo if k < 4 else ht_hi
                    nc.tensor.matmul(
                        rec[b][0:BL, :],
                        lhsT=src[:, 32 * (k % 4):32 * (k % 4) + 8],
                        rhs=u_sb[:, k, nbs],
                        start=False, stop=False, skip_group_check=True)

        if t + 1 < steps:
            rec_cur = emit_xp_copy(t + 1)

        # --- tanh ---
        # Output columns are written j-major permuted (logical col 128i+32j+b
        # lands at 128j+32i+b) so each stream-transpose above reads one
        # contiguous [32, 128] range. The host un-permutes (see assemble()).
        # Two separate tiles per bank (j-pairs) so the first transposes are
        # gated only by the first tanh instruction.
        halves = []
        for b in range(2):
            src = rec[b][:].rearrange("p (i j b) -> p i j b", i=4, j=4)
            sub = []
            for jj in range(2):
                hh_t = hpool.tile([32, 256], F32, tag=f"h{b}{jj}")
                nc.scalar.activation(
                    hh_t[:].rearrange("p (j i b) -> p i j b", j=2, i=4),
                    src[:, :, 2 * jj:2 * (jj + 1), :],
                    mybir.ActivationFunctionType.Tanh)
                sub.append(hh_t)
            halves.append(sub)

        # --- output DMA ---
        for b in range(2):
            for jj in range(2):
                nc.sync.dma_start(
                    out[BL * t:BL * (t + 1),
                        512 * b + 256 * jj:512 * b + 256 * (jj + 1)],
                    halves[b][jj][0:BL, :])

        prev_halves = halves


def build_nc(steps=S, enable_asserts=False):
    nc = bacc.Bacc("TRN2", target_bir_lowering=False, debug=False,
                   enable_asserts=enable_asserts)
    xT = nc.dram_tensor("xT", [I, steps * BL], F32R, kind="ExternalInput").ap()
    w = nc.dram_tensor("w", [I, H], F32R, kind="ExternalInput").ap()
    u = nc.dram_tensor("u", [H, H], F32R, kind="ExternalInput").ap()
    bias = nc.dram_tensor("bias", [1, H], F32R, kind="ExternalInput").ap()
    out = nc.dram_tensor("out", [steps * BL, H], F32, kind="ExternalOutput").ap()
    with tile.TileContext(nc) as tc:
        with ExitStack() as ctx:
            _emit_body(ctx, tc, xT, w, u, bias, out, steps)
    nc.compile()
    return nc


def round_f32r(a):
    """Round fp32 to the FP32R format (11 mantissa bits, RNE, low 12 bits 0)."""
    u = np.ascontiguousarray(a, dtype=np.float32).view(np.uint32)
    u = u + np.uint32(0x7FF) + ((u >> np.uint32(12)) & np.uint32(1))
    u &= np.uint32(0xFFFFF000)
    return u.view(np.float32)


def make_in_maps(x, W_ih_f, W_hh_f, b_ih_f, b_hh_f, W_ih_b, W_hh_b, b_ih_b, b_hh_b,
                 steps=S):
    """Build the 8 per-core input dicts. Cores 0-3 fwd, 4-7 bwd."""
    x = np.ascontiguousarray(np.asarray(x, dtype=np.float32)[:, :steps])
    sets = {
        "f": (np.asarray(W_ih_f), np.asarray(W_hh_f),
              np.asarray(b_ih_f) + np.asarray(b_hh_f)),
        "b": (np.asarray(W_ih_b), np.asarray(W_hh_b),
              np.asarray(b_ih_b) + np.asarray(b_hh_b)),
    }
    in_maps = []
    for core in range(NCORES):
        d = "f" if core < 4 else "b"
        g = core % 4
        Wih, Whh, bsum = sets[d]
        xs = x[BL * g:BL * (g + 1)]
        if d == "b":
            xs = xs[:, ::-1]
        # xT[i, s*BL + b] = xs[b, s, i]
        xT = np.ascontiguousarray(xs.transpose(2, 1, 0).reshape(I, steps * BL))
        in_maps.append({
            "xT": round_f32r(xT),
            "w": round_f32r(np.ascontiguousarray(Wih.T.astype(np.float32))),
            "u": round_f32r(np.ascontiguousarray(Whh.T.astype(np.float32))),
            "bias": round_f32r(np.ascontiguousarray(bsum.astype(np.float32)[None, :])),
        })
    return in_maps


def unpermute_out(o):
    """Undo the device-side j-major column permutation (per 512-col half)."""
    s = o.shape[:-1]
    return np.ascontiguousarray(
        o.reshape(*s, 2, 4, 4, 32).swapaxes(-3, -2).reshape(*s, H))


def assemble(results, steps=S):
    """results: list of 8 dicts with 'out' [steps*BL, H]. Returns [B, steps, 2H]."""
    full = np.empty((B, steps, 2 * H), dtype=np.float32)
    for core in range(NCORES):
        o = unpermute_out(np.asarray(results[core]["out"])).reshape(steps, BL, H)
        g = core % 4
        if core < 4:
            full[BL * g:BL * (g + 1), :, :H] = o.transpose(1, 0, 2)
        else:
            full[BL * g:BL * (g + 1), :, H:] = o[::-1].transpose(1, 0, 2)
    return full


def kernel(x, W_ih_f, W_hh_f, b_ih_f, b_hh_f, W_ih_b, W_hh_b, b_ih_b, b_hh_b):
    from concourse.bass_utils import run_bass_kernel_spmd
    nc = build_nc(S)
    in_maps = make_in_maps(x, W_ih_f, W_hh_f, b_ih_f, b_hh_f,
                           W_ih_b, W_hh_b, b_ih_b, b_hh_b)
    res = run_bass_kernel_spmd(nc, in_maps, list(range(NCORES))).results
    return assemble(res)
